# revision 1
# baseline (speedup 1.0000x reference)
"""SAGAN self-attention block on 8 TRN2 NeuronCores (v4).

Reference (per batch element b, N = H*W = 4096, C = 512, D = 64):
    f = x @ Wf + bf ; g = x @ Wg + bg ; h = x @ Wh + bh      # [N, D]
    s = f @ g.T                                              # [N, N]
    attn = softmax(s, axis=-1)
    ctx = attn @ h                                           # [N, D]
    o = (gamma * ctx) @ Wv + bv + x                          # [N, C]

Sharding: data-parallel over batch B=8 -> one batch element per core, no
collectives. Weights replicated.

Device algorithm (per core), matmuls in bf16 with f32 PSUM accumulation:
  - host passes xT (x transposed, bf16) so no on-device PE transposes;
    x rows (f32) are DMA'd just-in-time for the residual add only.
  - f and g projected in ONE matmul chain per 512-chunk using stacked
    [Wf|Wg] weights (M=128): fT lands on PSUM partitions 0:64, gT on
    64:128; DVE adds biases writing FT2/GT2 halves, which are then
    mirrored into the other partition half via SBUF->SBUF DMA so QK
    pairs can row-pack (K=64 streams 2 cols/cycle).
  - h_aug[m, :] = [x@Wh + bh, 1.0]  -> [4096, 65] bf16 (m on partitions).
  - unnormalized softmax (no max subtraction: |s| <~ 80 fits f32/bf16):
    m-tiles in groups of 3 -> one [128, 1536] PSUM tile, ONE wide EXP
    (ScalarE) per group -> SBUF bf16; PV accumulates ctxT[65, n-chunk]
    += haug[i].T @ ep (K=128); row 64 = softmax denominator (ones col).
  - epilogue per n-chunk: ct = bf16(ctxT); denominator row transposed to
    [128,1] via tiny SBUF->SBUF DMA; out = (ct.T @ [gamma*Wv ; bv]) *
    (1/den) + x fused in one DVE scalar_tensor_tensor.  The out-proj /
    store half of each epilogue is emitted AFTER the next chunk's first
    QK groups so the PE never idles at chunk boundaries; ctx and op
    tiles ring-share one 2-buffer PSUM pool.
  - prologue emission is interleaved with chunk-0 main groups so the
    QK/EXP pipeline starts ~3us in; xT loads are split per c-chunk
    across DMA queues.
"""

import numpy as np
import ml_dtypes

BF16 = ml_dtypes.bfloat16

B, HH, WW, C = 8, 64, 64, 512
D = C // 8          # 64
N_FULL = HH * WW    # 4096
P = 128
CC = C // P         # 4  (c-chunks of 128)

_CACHE: dict = {}


def _groups(n_tiles):
    """m-tile groups per n-chunk: triples + a final pair (e.g. 10x3 + 1x2)."""
    gs = []
    i = 0
    while n_tiles - i >= 3:
        if n_tiles - i == 4:
            break
        gs.append([i, i + 1, i + 2])
        i += 3
    while i < n_tiles:
        gs.append(list(range(i, min(i + 2, n_tiles))))
        i += 2
    return gs


def _build(n: int, h_bias_zero: bool = False):
    import concourse.mybir as mybir
    from concourse import bacc
    from concourse.tile import TileContext

    f32 = mybir.dt.float32
    bf16 = mybir.dt.bfloat16
    ADD = mybir.AluOpType.add
    MULT = mybir.AluOpType.mult
    EXP = mybir.ActivationFunctionType.Exp

    n_tiles = n // P        # 32
    nch = n // 512          # 8

    nc = bacc.Bacc("TRN2", target_bir_lowering=False, debug=False)

    x_d = nc.dram_tensor("x", [n, C], f32, kind="ExternalInput")
    xt_d = nc.dram_tensor("xt", [P, CC, n], bf16, kind="ExternalInput")
    wfg_d = nc.dram_tensor("wfg", [P, CC, 2 * D], bf16, kind="ExternalInput")
    wh_d = nc.dram_tensor("wh", [P, CC, D], bf16, kind="ExternalInput")
    bfg_d = nc.dram_tensor("bfg", [P, 1], f32, kind="ExternalInput")   # [bf;bg]
    if not h_bias_zero:
        bh_d = nc.dram_tensor("bhp", [1, D], bf16, kind="ExternalInput")
        on_d = nc.dram_tensor("onesp", [1, P], bf16, kind="ExternalInput")
    wv_d = nc.dram_tensor("wv", [D + 1, C], bf16, kind="ExternalInput")
    out_d = nc.dram_tensor("out", [n, C], f32, kind="ExternalOutput")

    x_t = x_d.rearrange("(i p) c -> i p c", p=P)
    o_t = out_d.rearrange("(i p) c -> i p c", p=P)

    groups = _groups(n_tiles)
    # prologue fg-chunk needed before main group g of chunk 0:
    need_fg = [(3 * g + 2) // 4 if len(grp) == 3 else (grp[-1] // 4)
               for g, grp in enumerate(groups)]

    with TileContext(nc) as tc:
        with (
            tc.tile_pool(name="const", bufs=1) as cpool,
            tc.tile_pool(name="big", bufs=1) as bigpool,
            tc.tile_pool(name="ep", bufs=5) as epool,
            tc.tile_pool(name="ct", bufs=2) as ctpool,
            tc.tile_pool(name="os", bufs=4) as opool,
            tc.tile_pool(name="xr", bufs=8) as xrpool,
            tc.tile_pool(name="sm", bufs=8) as smpool,
            tc.tile_pool(name="psA", bufs=2, space="PSUM") as psA,
            tc.tile_pool(name="psB", bufs=1, space="PSUM") as psB,
            tc.tile_pool(name="psC", bufs=1, space="PSUM") as psC,
        ):
            # ---- replicated constants -> SBUF
            wfg_sb = cpool.tile([P, CC, 2 * D], bf16)
            nc.sync.dma_start(wfg_sb, wfg_d[:, :, :])
            wh_sb = cpool.tile([P, CC, D], bf16)
            nc.sync.dma_start(wh_sb, wh_d[:, :, :])
            bfg_sb = cpool.tile([P, 1], f32)
            nc.sync.dma_start(bfg_sb, bfg_d[:, :])
            if not h_bias_zero:
                bh_sb = cpool.tile([1, D], bf16)
                nc.sync.dma_start(bh_sb, bh_d[:, :])
                ones_sb = cpool.tile([1, P], bf16)
                nc.sync.dma_start(ones_sb, on_d[:, :])
            wv_sb = cpool.tile([D + 1, C], bf16)
            nc.sync.dma_start(wv_sb, wv_d[:, :])

            # ---- persistent SBUF tensors
            xt = bigpool.tile([P, CC, n], bf16)          # x.T (c on partitions)
            FT2 = bigpool.tile([P, n], bf16)             # f.T duplicated in both halves
            GT2 = bigpool.tile([P, n], bf16)             # g.T duplicated in both halves
            haug = bigpool.tile([P, n_tiles, D + 1], bf16)
            nc.gpsimd.memset(haug[:, :, D:D + 1], 1.0)

            def emit_xt_dma(jc, split=1):
                for cc in range(CC):
                    for s in range(split):
                        w = 512 // split
                        sl = slice(jc * 512 + s * w, jc * 512 + (s + 1) * w)
                        nc.sync.dma_start(xt[:, cc, sl], xt_d[:, cc, sl])

            def emit_fg_chunk(jc):
                """f/g projection for 512-chunk jc."""
                sl = slice(jc * 512, (jc + 1) * 512)
                fgp = psA.tile([P, 512], f32, tag="sp", name=f"fg{jc}")
                for cc in range(CC):
                    nc.tensor.matmul(
                        fgp, lhsT=wfg_sb[:, cc, :], rhs=xt[:, cc, sl],
                        start=(cc == 0), stop=(cc == CC - 1),
                    )
                nc.vector.tensor_scalar(FT2[0:D, sl], fgp[0:D, :], bfg_sb[0:D], None, ADD)
                nc.vector.tensor_scalar(GT2[D:P, sl], fgp[D:P, :], bfg_sb[D:P], None, ADD)
                nc.sync.dma_start(FT2[D:P, sl], FT2[0:D, sl])
                nc.sync.dma_start(GT2[0:D, sl], GT2[D:P, sl])

            def emit_h_tile(i):
                """h projection + haug fill for m-tile i."""
                hp = psA.tile([P, D], f32, tag="sp", name=f"hp{i}")
                for cc in range(CC):
                    nc.tensor.matmul(
                        hp, lhsT=xt[:, cc, i * P:(i + 1) * P], rhs=wh_sb[:, cc, :],
                        start=(cc == 0), stop=(h_bias_zero and cc == CC - 1),
                    )
                if not h_bias_zero:
                    nc.tensor.matmul(
                        hp, lhsT=ones_sb, rhs=bh_sb, start=False, stop=True)
                nc.vector.tensor_copy(out=haug[:, i, 0:D], in_=hp)

            def emit_group(ck, g, ctx):
                """QK + EXP + PV for m-tile group g of column-chunk ck."""
                cs, cw, _ = ck
                sl = slice(cs, cs + cw)
                grp = groups[g]
                w = cw * len(grp)
                # each m-tile's QK output sits at a bank-aligned 512-col
                # slot; narrow chunks use a strided view for the single EXP
                sp = psA.tile([P, 1536], f32, tag="sp", name=f"sp{cs}_{g}")
                for q, i in enumerate(grp):
                    # row-pack QK by m-tile parity: even tiles read the lower
                    # halves of GT2/FT2, odd tiles the upper mirrors
                    hb = (i % 2) * D
                    nc.tensor.matmul(
                        sp[:, q * 512:q * 512 + cw],
                        lhsT=GT2[hb:hb + D, i * P:(i + 1) * P],
                        rhs=FT2[hb:hb + D, sl],
                        start=True, stop=True, tile_position=(hb, 0),
                    )
                ep = epool.tile([P, 1536], bf16, tag="ep")
                spv = sp.rearrange("p (q v) -> p q v", v=512)
                epv = ep.rearrange("p (q v) -> p q v", v=512)
                nc.scalar.activation(
                    epv[:, 0:len(grp), 0:cw], spv[:, 0:len(grp), 0:cw], EXP)
                for q, i in enumerate(grp):
                    nc.tensor.matmul(
                        ctx[:, 0:cw], lhsT=haug[:, i, :],
                        rhs=ep[:, q * 512:q * 512 + cw],
                        start=(g == 0 and q == 0), stop=(i == n_tiles - 1),
                    )

            def emit_ct_copy(ck, ctx):
                """ctx -> bf16 SBUF copy (DVE)."""
                cs, cw, _ = ck
                ct = ctpool.tile([D + 1, 512], bf16, tag="ct", name=f"ct{cs}")
                nc.vector.tensor_copy(out=ct[:, 0:cw], in_=ctx[:, 0:cw])
                return ct

            def emit_denoms(ck, ct):
                """PE transposes of the denominator row into one PSUM tile,
                then one batched DVE reciprocal -> rc4 [128, 2T] f32 SBUF."""
                _, cw, tiles = ck
                # bf16 PSUM writes need 4-byte alignment: space columns 2 apart
                dt4 = psC.tile([P, 8], bf16, tag="oc", name=f"dt{tiles[0]}")
                for t in range(len(tiles)):
                    tsl = slice(t * P, (t + 1) * P)
                    nc.tensor.transpose(
                        dt4[:, 2 * t:2 * t + 1], ct[D:D + 1, tsl],
                        haug[D:D + 1, 0, D:D + 1])
                rc4 = smpool.tile([P, 8], f32, tag="rc")
                nc.vector.reciprocal(rc4[:, 0:2 * len(tiles)], dt4[:, 0:2 * len(tiles)])
                return rc4

            def emit_out_tile(ck, t, ct, rc4, tail=False, pool=None):
                """out-proj + scale + residual + store for one 128-row tile."""
                it = ck[2][t]
                tsl = slice(t * P, (t + 1) * P)
                op = (pool or psC).tile([P, C], f32,
                                        tag="sp" if pool is psA else "oc",
                                        name=f"op{it}")
                nc.tensor.matmul(op, lhsT=ct[:, tsl], rhs=wv_sb, start=True, stop=True)
                osb = opool.tile([P, C], f32, tag="os")
                nc.vector.scalar_tensor_tensor(
                    out=osb, in0=op, scalar=rc4[:, 2 * t:2 * t + 1], in1=xrs_of[it],
                    op0=MULT, op1=ADD)
                if tail:
                    # final stores are the kernel tail: 4-way queue split
                    for q in range(4):
                        qs = slice(q * 32, (q + 1) * 32)
                        nc.sync.dma_start(o_t[it][qs, :], osb[qs, :])
                else:
                    # split mid-run stores too: halves the per-queue backlog
                    nc.sync.dma_start(o_t[it][0:D, :], osb[0:D, :])
                    nc.sync.dma_start(o_t[it][D:P, :], osb[D:P, :])

            # ---- emission schedule -------------------------------------
            # column chunks: full 512-wide ones, then the last 512 columns as
            # two 256-wide minis so their stores overlap remaining compute
            chunks = [(j * 512, 512, [4 * j + t for t in range(4)])
                      for j in range(nch - 1)]
            chunks.append((n - 512, 256, [n_tiles - 4, n_tiles - 3]))
            chunks.append((n - 256, 256, [n_tiles - 2, n_tiles - 1]))

            emit_xt_dma(0, split=2)
            emit_xt_dma(1)
            xt_done = 2
            fg_done = 0
            h_done = 0
            xrs_of = {}
            pending = None
            for ci, ck in enumerate(chunks):
                cs, cw, tiles = ck
                first = (ci == 0)
                is_tail = (ci >= len(chunks) - 2)
                for it in tiles:
                    xr = xrpool.tile([P, C], f32, tag="xr", name=f"xr{it}")
                    nc.sync.dma_start(xr, x_t[it])
                    xrs_of[it] = xr
                ctx = psB.tile([D + 1, 512], f32, tag="cx", name=f"ctx{cs}")
                rc4p = None
                for g, grp in enumerate(groups):
                    if first:
                        # prologue production rides just ahead of consumption
                        while fg_done <= min(need_fg[g] + 1, nch - 1):
                            if xt_done < nch:
                                emit_xt_dma(xt_done)
                                xt_done += 1
                            emit_fg_chunk(fg_done)
                            fg_done += 1
                        while h_done < 4 * fg_done and h_done <= grp[-1] + 5:
                            emit_h_tile(h_done)
                            h_done += 1
                    emit_group(ck, g, ctx)
                    if pending is not None:
                        pck, pct = pending
                        if g == 0:
                            rc4p = emit_denoms(pck, pct)
                        elif g <= len(pck[2]):
                            emit_out_tile(pck, g - 1, pct, rc4p,
                                          tail=(ci >= len(chunks) - 1))
                            if g == len(pck[2]):
                                pending = None
                pending = (ck, emit_ct_copy(ck, ctx))
            pck, pct = pending
            rc4p = emit_denoms(pck, pct)
            for t in range(len(pck[2])):
                emit_out_tile(pck, t, pct, rc4p, tail=True,
                              pool=(psA if t % 2 else psC))

    nc.compile()
    return nc


def get_program(n: int = N_FULL, h_bias_zero: bool = False):
    key = (n, h_bias_zero)
    if key not in _CACHE:
        _CACHE[key] = _build(n, h_bias_zero)
    return _CACHE[key]


def make_weight_maps(Wf, bf, Wg, bg, Wh, bh, Wv, bv, gamma, h_bias_zero=False):
    """Host-side layout prep of the tiny replicated weights."""
    wv_aug = np.concatenate(
        [np.float32(gamma) * np.asarray(Wv, np.float32),
         np.asarray(bv, np.float32)[None, :]], axis=0)
    bfg = np.concatenate(
        [np.asarray(bf, np.float32), np.asarray(bg, np.float32)]).reshape(P, 1)
    wfg = np.concatenate(
        [np.asarray(Wf, np.float32), np.asarray(Wg, np.float32)], axis=1)
    # c index decomposition: c = cc*128 + p  ->  [p, cc, d]
    maps = {
        "wfg": np.ascontiguousarray(
            wfg.astype(BF16).reshape(CC, P, 2 * D).transpose(1, 0, 2)),
        "wh": np.ascontiguousarray(
            np.asarray(Wh, np.float32).astype(BF16).reshape(CC, P, D).transpose(1, 0, 2)),
        "bfg": np.ascontiguousarray(bfg),
        "bhp": np.ascontiguousarray(
            np.asarray(bh, np.float32).astype(BF16).reshape(1, D)),
        "onesp": np.ones((1, P), dtype=BF16),
        "wv": np.ascontiguousarray(wv_aug.astype(BF16)),
    }
    if h_bias_zero:
        del maps["bhp"], maps["onesp"]
    return maps


def make_x_maps(xf_b):
    """Per-core x layouts: residual rows (f32) + transposed bf16 [p, cc, n]."""
    x = np.ascontiguousarray(xf_b, dtype=np.float32)
    xt = x.T.astype(BF16).reshape(CC, P, x.shape[0]).transpose(1, 0, 2)
    return {"x": x, "xt": np.ascontiguousarray(xt)}


def kernel(x, Wf, bf, Wg, bg, Wh, bh, Wv, bv, gamma):
    from concourse.bass_utils import run_bass_kernel_spmd

    x = np.asarray(x, np.float32)
    b, hh, ww, c = x.shape
    n = hh * ww
    assert (b, c) == (B, C)

    hbz = bool(np.all(np.asarray(bh) == 0))
    nc = get_program(n, hbz)
    base = make_weight_maps(Wf, bf, Wg, bg, Wh, bh, Wv, bv, gamma, hbz)
    xf = x.reshape(b, n, c)
    in_maps = [dict(base, **make_x_maps(xf[i])) for i in range(b)]

    res = run_bass_kernel_spmd(nc, in_maps, core_ids=list(range(b)))
    out = np.stack([res.results[i]["out"] for i in range(b)], axis=0)
    return np.ascontiguousarray(out.reshape(b, hh, ww, c).astype(np.float32))

